# revision 1
# baseline (speedup 1.0000x reference)
"""AdaptiveSpectrumLayer Trainium2 kernel — 8-core pure data parallel, v2.

Pipeline per core (B_local=8 batches, COLS=1024 columns = (b,f)):
  rfft (fp32r DFT matmuls, fp32 PSUM accumulate)
  -> mag/s/c features in bf16 (DVE 2x ops; no trig: s=im/mag, c=re/mag)
  -> per-freq 4->32->2 relu MLP (block-diag bf16 matmuls; bias folded
     into matmul via constant ones-row in ff; relus split Act/DVE/Pool)
  -> reduce into paired [128x1024] PSUM tiles (2 freq groups/tile),
     full-width relu/sigmoid, DMA extraction of m/ph
  -> gate collapsed to three 257x257 matmuls (bias via ones-row)
     -> swish -> sigmoid weights (bf16)
  -> spectrum blend in bf16 (DVE 2x) -> irfft (bf16 DFT matmuls),
     output DMA'd straight from PSUM.
"""
import sys
import numpy as np

sys.path.insert(0, "/opt/trn_rl_repo")

import ml_dtypes
from contextlib import ExitStack

import concourse.bass as bass
import concourse.tile as tile
from concourse import mybir
from concourse import bacc
from concourse.bass_utils import run_bass_kernel_spmd


def _ensure_ntff_hook():
    """The agent image's antenv lacks axon_hooks; inject a stub and register
    the ctypes NTFF profiling hook so trace=True works. Safe no-op if parts
    are missing."""
    try:
        import antenv.axon_hooks  # noqa: F401
        return
    except ImportError:
        pass
    try:
        import types
        import antenv
        mod = types.ModuleType("antenv.axon_hooks")
        _state = {"hook": None}
        mod.set_axon_ntff_profile_hook = lambda h: _state.__setitem__("hook", h)
        mod.get_axon_ntff_profile_hook = lambda: _state["hook"]
        sys.modules["antenv.axon_hooks"] = mod
        antenv.axon_hooks = mod
        from trn_agent_boot.trn_boot import _ntff_profile_via_ctypes
        so = "/opt/axon/libaxon_pjrt.so"
        import os
        if os.path.exists(so):
            mod.set_axon_ntff_profile_hook(_ntff_profile_via_ctypes(so))
    except Exception:
        pass


_ensure_ntff_hook()

# ---- problem constants (hardcoded; kernel.py must be self-contained) ----
B, H, F, HID = 64, 512, 128, 32
FS = 100.0
NF = H // 2 + 1          # 257
NFP = 288                # padded freq count: 9 groups of 32 = 3 chunks of 96
NG = 9                   # freq groups (32 each)
NCH = 3                  # freq chunks (96 each)
CPW = 96                 # chunk width
NCORE = 8
BL = B // NCORE          # 8
COLS = BL * F            # 1024
NC2 = 2                  # 512-wide N chunks per 1024 cols
NPAIR = 5                # reduce pairs: (0,1)(2,3)(4,5)(6,7)(8)
EPS = 1e-30

f32 = mybir.dt.float32
f32r = mybir.dt.float32r
bf16 = mybir.dt.bfloat16
AF = mybir.ActivationFunctionType
ALU = mybir.AluOpType


# =========================================================================
# Host-side weight preparation
# =========================================================================
def build_host_weights(Wp, bp, Wg, bg, Wm, bm, Wph, bph):
    freqs = np.fft.rfftfreq(H, 1.0 / FS)[:NF].astype(np.float32)

    n_idx = np.arange(NFP)
    t_idx = np.arange(H)
    valid = (n_idx < NF).astype(np.float32)
    theta = 2.0 * np.pi * np.outer(t_idx, n_idx) / H  # (512, 288)
    inv_sqrt_h = 1.0 / np.sqrt(H)

    RC = (np.cos(theta) * inv_sqrt_h * valid[None, :]).astype(np.float32)
    RS = (-np.sin(theta) * inv_sqrt_h * valid[None, :]).astype(np.float32)
    w_n = np.where((n_idx == 0) | (n_idx == 256), 1.0, 2.0) * valid
    IC = (np.cos(theta) * inv_sqrt_h * w_n[None, :]).astype(np.float32)
    IS = (-np.sin(theta) * inv_sqrt_h * w_n[None, :]).astype(np.float32)

    WpP = np.zeros((NFP, 4, HID), np.float32); WpP[:NF] = Wp
    bpP = np.zeros((NFP, HID), np.float32);    bpP[:NF] = bp
    WmP = np.zeros((NFP, HID), np.float32);    WmP[:NF] = Wm
    bmP = np.zeros((NFP,), np.float32);        bmP[:NF] = bm
    WphP = np.zeros((NFP, HID), np.float32);   WphP[:NF] = Wph
    bphP = np.zeros((NFP,), np.float32);       bphP[:NF] = bph
    fP = np.zeros((NFP,), np.float32);         fP[:NF] = freqs

    D = fP[:, None] * WpP[:, 3, :] + bpP  # (288, 32)

    WgR = Wg.reshape(NF, HID, NF)
    G = np.zeros((3, NFP, NFP), np.float32)
    for f in range(3):
        G[f, :NF, :NF] = np.einsum("nh,nhj->nj", Wp[:, f, :], WgR)
    gconst = np.zeros((NFP,), np.float32)
    gconst[:NF] = np.einsum("nh,nhj->j", D[:NF], WgR) + bg

    # ---- device layouts ----
    # w_rfft (128, 4, 2, 3, 96) bf16: [tp][kt][ri][ch][fc]
    w_rfft = np.zeros((128, 4, 2, NCH, CPW), np.float32)
    RCr = RC.reshape(4, 128, NCH, CPW)  # [kt][tp][ch][fc]
    RSr = RS.reshape(4, 128, NCH, CPW)
    w_rfft[:, :, 0] = RCr.transpose(1, 0, 2, 3)
    w_rfft[:, :, 1] = RSr.transpose(1, 0, 2, 3)

    # w_proj (128, 9, 8, 128) bf16; row 96 carries D (ff row 96 == 1)
    w_proj = np.zeros((128, NG, 8, 128), np.float32)
    ii = np.arange(32)
    for g in range(NG):
        n = 32 * g + ii  # (32,)
        for f in range(3):
            feat = WpP[n, f, :]  # (32, 32) [i, h]
            for j in range(8):
                blk = feat[:, 4 * j:4 * j + 4]  # (32 i, 4 hh)
                for hh in range(4):
                    w_proj[32 * f + ii, g, j, 4 * ii + hh] = blk[:, hh]
        for j in range(8):
            for hh in range(4):
                w_proj[96, g, j, 4 * ii + hh] = D[n, 4 * j + hh]

    # w_red (128, 9, 8, 64): [4i+hh][g][j][col]
    w_red = np.zeros((128, NG, 8, 64), np.float32)
    for g in range(NG):
        n = 32 * g + ii
        for j in range(8):
            for hh in range(4):
                w_red[4 * ii + hh, g, j, ii] = WmP[n, 4 * j + hh]
                w_red[4 * ii + hh, g, j, 32 + ii] = WphP[n, 4 * j + hh]

    # compressed group 8: only n=256 is a real frequency.
    # w_proj8 (97, 32): contract rows {0:mag, 32:sin, 64:cos, 96:one} -> hid
    w_proj8 = np.zeros((97, 32), np.float32)
    for f in range(3):
        w_proj8[32 * f, :] = WpP[256, f, :]
    w_proj8[96, :] = D[256, :]
    # w_red8 (32, 2): hid -> [m, ph] for n=256
    w_red8 = np.zeros((32, 2), np.float32)
    w_red8[:, 0] = WmP[256, :]
    w_red8[:, 1] = WphP[256, :]

    # mp_bias2 (128, 5): pair p rows = [bm g | bph g | bm g' | bph g']
    mp_bias2 = np.zeros((128, NPAIR), np.float32)
    for p in range(NPAIR - 1):
        g, g2 = 2 * p, 2 * p + 1
        mp_bias2[0:32, p] = bmP[32 * g + ii]
        mp_bias2[32:64, p] = bphP[32 * g + ii]
        mp_bias2[64:96, p] = bmP[32 * g2 + ii]
        mp_bias2[96:128, p] = bphP[32 * g2 + ii]
    # last pair: compressed g8 -> rows 0/1 = bm/bph of n=256
    mp_bias2[0, NPAIR - 1] = bmP[256]
    mp_bias2[1, NPAIR - 1] = bphP[256]

    # w_gate (128, 9, 3, 96): [32f+i][g][jt][jc]; row 96 of g=0 carries gconst
    w_gate = np.zeros((128, NG, NCH, CPW), np.float32)
    for g in range(NG):
        n = 32 * g + ii
        for f in range(3):
            Gr = G[f][n].reshape(32, NCH, CPW)  # [i][jt][jc]
            w_gate[32 * f + ii, g] = Gr
    w_gate[96, 0] = gconst.reshape(NCH, CPW)

    # w_irfft (96, 2, 3, 4, 128): [p][ri][ch][mt][tc]
    w_irfft = np.zeros((CPW, 2, NCH, 4, 128), np.float32)
    ICr = IC.reshape(4, 128, NCH, CPW)  # [mt][tc][ch][p]
    ISr = IS.reshape(4, 128, NCH, CPW)
    w_irfft[:, 0] = ICr.transpose(3, 2, 0, 1)
    w_irfft[:, 1] = ISr.transpose(3, 2, 0, 1)

    tobf = lambda a: a.astype(ml_dtypes.bfloat16)
    return dict(
        w_rfft=tobf(w_rfft),
        w_proj=tobf(w_proj),
        w_red=tobf(w_red),
        mp_bias2=mp_bias2,
        w_gate=tobf(w_gate),
        w_irfft=tobf(w_irfft),
        ones=np.ones((1, NG, COLS), np.float32).astype(ml_dtypes.bfloat16),
        w_proj8=tobf(w_proj8),
        w_red8=tobf(w_red8),
    )


# =========================================================================
# Device kernel builder
# =========================================================================
def build_kernel():
    nc = bacc.Bacc()

    x_d = nc.declare_dram_parameter("x", [128, 4, BL, F], bf16, isOutput=False)
    w_rfft_d = nc.declare_dram_parameter("w_rfft", [128, 4, 2, NCH, CPW], bf16, isOutput=False)
    w_proj_d = nc.declare_dram_parameter("w_proj", [128, NG, 8, 128], bf16, isOutput=False)
    w_red_d = nc.declare_dram_parameter("w_red", [128, NG, 8, 64], bf16, isOutput=False)
    mp_bias2_d = nc.declare_dram_parameter("mp_bias2", [128, NPAIR], f32, isOutput=False)
    w_gate_d = nc.declare_dram_parameter("w_gate", [128, NG, NCH, CPW], bf16, isOutput=False)
    w_irfft_d = nc.declare_dram_parameter("w_irfft", [CPW, 2, NCH, 4, 128], bf16, isOutput=False)
    ones_d = nc.declare_dram_parameter("ones", [1, NG, COLS], bf16, isOutput=False)
    w_proj8_d = nc.declare_dram_parameter("w_proj8", [97, 32], bf16, isOutput=False)
    w_red8_d = nc.declare_dram_parameter("w_red8", [32, 2], bf16, isOutput=False)
    out_d = nc.declare_dram_parameter("out", [128, 4, BL, F], f32, isOutput=True)

    TWO_PI = float(2.0 * np.pi)
    PI = float(np.pi)

    with tile.TileContext(nc) as tc, ExitStack() as ctx:
        consts = ctx.enter_context(tc.tile_pool(name="consts", bufs=1))
        scratch = ctx.enter_context(tc.tile_pool(name="scratch", bufs=1))
        xr_pool = ctx.enter_context(tc.tile_pool(name="xr", bufs=12))
        rs_pool = ctx.enter_context(tc.tile_pool(name="rs", bufs=2))
        gt_pool = ctx.enter_context(tc.tile_pool(name="gt", bufs=2))
        bl_pool = ctx.enter_context(tc.tile_pool(name="bl", bufs=1))

        ps_proj = ctx.enter_context(tc.tile_pool(name="ps_proj", bufs=3, space="PSUM"))
        ps_red = ctx.enter_context(tc.tile_pool(name="ps_red", bufs=1, space="PSUM"))
        ps_misc = ctx.enter_context(tc.tile_pool(name="ps_misc", bufs=3, space="PSUM"))

        # ---- persistent SBUF tensors ----
        x_sb = consts.tile([128, 4, BL, F], bf16, tag="x_in")
        w_rfft_sb = consts.tile([128, 4, 2, NCH, CPW], bf16, tag="w_rfft")
        w_proj_sb = consts.tile([128, NG, 8, 128], bf16, tag="w_proj")
        w_red_sb = consts.tile([128, NG, 8, 64], bf16, tag="w_red")
        mp_bias2_sb = consts.tile([128, NPAIR], f32, tag="mp_bias2")
        w_gate_sb = consts.tile([128, NG, NCH, CPW], bf16, tag="w_gate")
        w_irfft_sb = consts.tile([CPW, 2, NCH, 4, 128], bf16, tag="w_irfft")

        re_sb = consts.tile([CPW, NCH, COLS], bf16, tag="re")
        im_sb = consts.tile([CPW, NCH, COLS], bf16, tag="im")
        ff_sb = consts.tile([97, NG, COLS], bf16, tag="ff")
        m_sb = consts.tile([CPW, NCH, COLS], bf16, tag="m_t")
        ph_sb = consts.tile([CPW, NCH, COLS], f32, tag="ph_t")
        w_sb = consts.tile([CPW, NCH, COLS], bf16, tag="w_t")
        sin_sb = consts.tile([CPW, NCH, COLS], bf16, tag="sin_t")
        cos_sb = consts.tile([CPW, NCH, COLS], bf16, tag="cos_t")

        # ---- load weights + input (spread across DGE queues by need-time) ----
        nc.gpsimd.dma_start(out=w_rfft_sb, in_=w_rfft_d[:])
        nc.sync.dma_start(out=x_sb[:, 0:2], in_=x_d[:, 0:2])
        nc.sync.dma_start(out=x_sb[:, 2:4], in_=x_d[:, 2:4])
        nc.gpsimd.dma_start(out=w_proj_sb[:, 0:3], in_=w_proj_d[:, 0:3])
        # ff row 96 := 1.0 (bias path for proj/gate; rows 0..95 DMA'd later)
        nc.gpsimd.dma_start(out=ff_sb[96:97, :, :], in_=ones_d[:])
        nc.gpsimd.dma_start(out=mp_bias2_sb, in_=mp_bias2_d[:])
        w_proj8_sb = consts.tile([97, 32], bf16, tag="w_proj8")
        w_red8_sb = consts.tile([32, 2], bf16, tag="w_red8")
        nc.gpsimd.dma_start(out=w_proj8_sb, in_=w_proj8_d[:])
        nc.gpsimd.dma_start(out=w_red8_sb, in_=w_red8_d[:])
        # padding rows of chunk 2 (freqs 257..287): m must be 0 (so the
        # blend passes fft through) and ph must be finite
        nc.vector.memset(m_sb[64:96, 2, :], 0.0)
        nc.vector.memset(ph_sb[64:96, 2, :], 0.0)
        nc.gpsimd.dma_start(out=w_proj_sb[:, 3:9], in_=w_proj_d[:, 3:9])
        nc.scalar.dma_start(out=w_red_sb, in_=w_red_d[:])
        nc.scalar.dma_start(out=w_gate_sb, in_=w_gate_d[:])
        nc.scalar.dma_start(out=w_irfft_sb, in_=w_irfft_d[:])

        def const_col(value, tag):
            t = consts.tile([128, 1], f32, tag=tag)
            nc.vector.memset(t, value)
            return t

        eps_c = const_col(EPS, "c_eps")
        pi_c = const_col(PI, "c_pi")

        # ================= rfft + features, per chunk =================
        def rfft_chunk(ch):
            for ri in range(2):
                dst = re_sb if ri == 0 else im_sb
                pts = [ps_misc.tile([128, 512], f32, tag="ps_misc",
                                    name="ps_misc")[:CPW] for _ in range(NC2)]
                for kt in range(4):
                    for nck in range(NC2):
                        nc.tensor.matmul(
                            out=pts[nck],
                            lhsT=w_rfft_sb[:, kt, ri, ch, :],
                            rhs=x_sb[:, kt, 4 * nck:4 * (nck + 1), :],
                            start=(kt == 0),
                            stop=(kt == 3),
                        )
                for nck in range(NC2):
                    # PSUM fp32 -> SBUF bf16
                    nc.vector.tensor_copy(
                        out=dst[:, ch, 512 * nck:512 * (nck + 1)], in_=pts[nck]
                    )

        def features_chunk(ch):
            re_c = re_sb[:, ch, :]
            im_c = im_sb[:, ch, :]
            msq = scratch.tile([CPW, COLS], bf16, tag="msq")
            t2 = scratch.tile([CPW, COLS], bf16, tag="tmpb")
            nc.vector.tensor_mul(out=msq, in0=re_c, in1=re_c)
            nc.vector.tensor_mul(out=t2, in0=im_c, in1=im_c)
            nc.vector.tensor_add(out=msq, in0=msq, in1=t2)
            magf = scratch.tile([CPW, COLS], f32, tag="magf")
            nc.scalar.activation(out=magf, in_=msq, func=AF.Sqrt,
                                 bias=eps_c[:CPW], scale=1.0)
            rr = scratch.tile([CPW, COLS], f32, tag="rr")
            nc.vector.reciprocal_approx_fast(out=rr, in_=magf)
            magb = scratch.tile([CPW, COLS], bf16, tag="magb")
            nc.scalar.copy(out=magb, in_=magf)
            sbf = scratch.tile([CPW, COLS], bf16, tag="sbf")
            cbf = scratch.tile([CPW, COLS], bf16, tag="cbf")
            nc.vector.tensor_mul(out=sbf, in0=im_c, in1=rr)
            nc.vector.tensor_mul(out=cbf, in0=re_c, in1=rr)
            # interleave into FF via SBUF->SBUF DMA (cross-partition)
            for q in range(3):  # groups 3*ch + q, rows 32q..32q+32
                g = 3 * ch + q
                for f, srct in enumerate((magb, sbf, cbf)):
                    eng = nc.sync if (q * 3 + f) % 2 == 0 else nc.scalar
                    eng.dma_start(
                        out=ff_sb[32 * f:32 * f + 32, g, :],
                        in_=srct[32 * q:32 * q + 32, :])

        rfft_chunk(0)
        rfft_chunk(1)
        features_chunk(0)
        rfft_chunk(2)
        features_chunk(1)
        features_chunk(2)

        # ================= per-freq MLP =================
        # relu engine pattern per (j, nck): A=scalar, D=vector
        RELU_PAT = "ADADADAD" "ADADADAA"

        pair_tiles = {}

        def mlp_group8():
            # compressed last group: only n=256 is real -> 32-wide MLP
            pair_tiles[4] = ps_red.tile([128, 1024], f32, tag="ps_red",
                                        name="ps_red")
            pair_pt = pair_tiles[4]
            for nck in range(NC2):
                proj_pt = ps_proj.tile([128, 512], f32, tag="ps_proj",
                                       name="ps_proj")
                nc.tensor.matmul(
                    out=proj_pt[:32],
                    lhsT=w_proj8_sb,
                    rhs=ff_sb[:, 8, 512 * nck:512 * (nck + 1)],
                    start=True, stop=True,
                )
                xr = xr_pool.tile([128, 512], bf16, tag="xr")
                if nck == 0:
                    nc.scalar.activation(out=xr[:32], in_=proj_pt[:32],
                                         func=AF.Relu, bias=0.0, scale=1.0)
                else:
                    nc.vector.tensor_scalar(out=xr[:32], in0=proj_pt[:32],
                                            scalar1=0.0, scalar2=None,
                                            op0=ALU.max)
                nc.tensor.matmul(
                    out=pair_pt[0:2, 512 * nck:512 * (nck + 1)],
                    lhsT=w_red8_sb,
                    rhs=xr[:32],
                    start=True, stop=True,
                )

        def mlp_group(g):
            p = g // 2
            if g % 2 == 0:
                pair_tiles[p] = ps_red.tile([128, 1024], f32, tag="ps_red",
                                            name="ps_red")
            pair_pt = pair_tiles[p]
            r0 = 64 * (g % 2)
            xrs = []
            k = 0
            for j in range(8):
                xr2 = []
                for nck in range(NC2):
                    proj_pt = ps_proj.tile([128, 512], f32, tag="ps_proj",
                                           name="ps_proj")
                    nc.tensor.matmul(
                        out=proj_pt,
                        lhsT=w_proj_sb[:97, g, j, :],
                        rhs=ff_sb[:, g, 512 * nck:512 * (nck + 1)],
                        start=True, stop=True,
                    )
                    xr = xr_pool.tile([128, 512], bf16, tag="xr")
                    eng = RELU_PAT[k]
                    k += 1
                    if eng == "A":
                        nc.scalar.activation(out=xr, in_=proj_pt, func=AF.Relu,
                                             bias=0.0, scale=1.0)
                    elif eng == "D":
                        nc.vector.tensor_scalar(
                            out=xr, in0=proj_pt, scalar1=0.0, scalar2=None,
                            op0=ALU.max)
                    else:
                        nc.gpsimd.tensor_scalar(
                            out=xr, in0=proj_pt, scalar1=0.0, scalar2=None,
                            op0=ALU.max)
                    xr2.append(xr)
                xrs.append(xr2)
            for j in range(8):
                for nck in range(NC2):
                    nc.tensor.matmul(
                        out=pair_pt[r0:r0 + 64, 512 * nck:512 * (nck + 1)],
                        lhsT=w_red_sb[:, g, j, :],
                        rhs=xrs[j][nck],
                        start=(j == 0), stop=(j == 7),
                    )

        def pair_acts(p):
            pair_pt = pair_tiles[p]
            R = rs_pool.tile([128, 1024], bf16, tag="mpR")
            S = rs_pool.tile([128, 1024], f32, tag="mpS")
            if p == NPAIR - 1:
                # compressed g8 pair: rows 0/1 = m/ph of n=256; relu on DVE
                # so it runs parallel to the sigmoid on Act
                nc.vector.tensor_scalar(
                    out=R[0:1], in0=pair_pt[0:1],
                    scalar1=mp_bias2_sb[0:1, p:p + 1], scalar2=0.0,
                    op0=ALU.add, op1=ALU.max)
                nc.scalar.activation(out=S[0:2], in_=pair_pt[0:2],
                                     func=AF.Sigmoid,
                                     bias=mp_bias2_sb[0:2, p:p + 1],
                                     scale=1.0)
                nc.gpsimd.dma_start(out=m_sb[64:65, 2, :], in_=R[0:1, :])
                nc.gpsimd.dma_start(out=ph_sb[64:65, 2, :], in_=S[1:2, :])
                return
            nc.scalar.activation(
                out=R, in_=pair_pt, func=AF.Relu,
                bias=mp_bias2_sb[:, p:p + 1], scale=1.0)
            nc.scalar.activation(out=S, in_=pair_pt,
                                 func=AF.Sigmoid,
                                 bias=mp_bias2_sb[:, p:p + 1], scale=1.0)
            for gg in (2 * p, 2 * p + 1):
                r0 = 64 * (gg % 2)
                ch, p0 = gg // 3, 32 * (gg % 3)
                nc.gpsimd.dma_start(out=m_sb[p0:p0 + 32, ch, :],
                                    in_=R[r0:r0 + 32, :])
                nc.gpsimd.dma_start(out=ph_sb[p0:p0 + 32, ch, :],
                                    in_=S[r0 + 32:r0 + 64, :])

        def trig_chunk(ch, r0=0, r1=CPW):
            rs = slice(r0, r1)
            ph_c = ph_sb[rs, ch, :]
            shalf = bl_pool.tile([CPW, COLS], f32, tag="shalf")
            # Sin spline valid on [-pi, pi]:
            # sin(2pi u) = sin(pi - 2pi u); cos(2pi u) = 1 - 2 sin^2(pi u)
            nc.scalar.activation(out=shalf[rs], in_=ph_c, func=AF.Sin,
                                 bias=0.0, scale=PI)
            nc.scalar.activation(out=sin_sb[rs, ch, :], in_=ph_c, func=AF.Sin,
                                 bias=pi_c[rs], scale=-TWO_PI)
            sh2 = bl_pool.tile([CPW, COLS], bf16, tag="sh2")
            nc.vector.tensor_mul(out=sh2[rs], in0=shalf[rs], in1=shalf[rs])
            nc.vector.tensor_scalar(out=cos_sb[rs, ch, :], in0=sh2[rs],
                                    scalar1=-2.0, scalar2=1.0,
                                    op0=ALU.mult, op1=ALU.add)

        def gate_jt(jt):
            gps = [ps_misc.tile([128, 512], f32, tag="ps_misc",
                                name="ps_misc")[:CPW] for _ in range(NC2)]
            for g in range(NG):
                for nck in range(NC2):
                    nc.tensor.matmul(
                        out=gps[nck],
                        lhsT=w_gate_sb[:97, g, jt, :],
                        rhs=ff_sb[:, g, 512 * nck:512 * (nck + 1)],
                        start=(g == 0), stop=(g == NG - 1),
                    )
            gt = gt_pool.tile([CPW, COLS], bf16, tag="gt")
            for nck in range(NC2):
                cs = slice(512 * nck, 512 * (nck + 1))
                sg = gt_pool.tile([CPW, 512], bf16, tag="sg")
                nc.scalar.activation(out=sg, in_=gps[nck], func=AF.Sigmoid,
                                     bias=0.0, scale=1.0)
                # gt = (gp + 0) * sigmoid(gp)  (swish)
                nc.vector.scalar_tensor_tensor(
                    out=gt[:, cs], in0=gps[nck], scalar=0.0, in1=sg,
                    op0=ALU.add, op1=ALU.mult)
            nc.scalar.activation(out=w_sb[:, jt, :], in_=gt, func=AF.Sigmoid,
                                 bias=0.0, scale=1.0)

        def blend_chunk(ch, r0=0, r1=CPW):
            rs = slice(r0, r1)
            m_c = m_sb[rs, ch, :]
            w_c = w_sb[rs, ch, :]
            u = bl_pool.tile([CPW, COLS], bf16, tag="u_t")
            wm = bl_pool.tile([CPW, COLS], bf16, tag="wm_t")
            nc.vector.tensor_scalar(out=u[rs], in0=w_c, scalar1=-1.0,
                                    scalar2=1.0, op0=ALU.mult, op1=ALU.add)
            nc.vector.tensor_mul(out=wm[rs], in0=w_c, in1=m_c)
            for trig, dst in ((cos_sb, re_sb), (sin_sb, im_sb)):
                a = bl_pool.tile([CPW, COLS], bf16, tag="a_t")
                b = bl_pool.tile([CPW, COLS], bf16, tag="b_t")
                nc.vector.tensor_mul(out=a[rs], in0=wm[rs],
                                     in1=trig[rs, ch, :])
                nc.vector.tensor_mul(out=b[rs], in0=u[rs], in1=dst[rs, ch, :])
                nc.vector.tensor_add(out=dst[rs, ch, :], in0=a[rs], in1=b[rs])

        # MLP with pair acts / trig / gate / blend interleaved.
        # pair/trig/blend run at high scheduler priority: they form the
        # latency-critical chain feeding the blends and irfft.
        # g8 (compressed, freqs 256+) runs BEFORE g6/g7 so the ch2 rows
        # 64..95 trig/blend leave the critical tail; only pair3 (rows 0..63)
        # remains after the last big group.
        for g in (0, 1, 2, 3, 4, 5, 8, 6, 7):
            if g == 8:
                mlp_group8()
                with tc.high_priority():
                    pair_acts(4)
                continue
            mlp_group(g)
            if g % 2 == 1:
                with tc.high_priority():
                    pair_acts(g // 2)
            if g == 4:
                gate_jt(0)
            if g == 5:
                with tc.high_priority():
                    trig_chunk(0)
                    trig_chunk(1)
                    blend_chunk(0)
            if g == 6:
                gate_jt(1)
                with tc.high_priority():
                    blend_chunk(1)
                gate_jt(2)
                with tc.high_priority():
                    trig_chunk(2, 64, CPW)
                    blend_chunk(2, 64, CPW)
        with tc.high_priority():
            trig_chunk(2, 0, 64)
            blend_chunk(2, 0, 64)

        # ================= irfft (two passes) =================
        # pass A: ch0+ch1 contributions -> bf16 partials (runs as soon as
        # blends 0/1 land); pass B (tail): 2 ch2 matmuls + fused add.
        out_sb = consts.tile([128, 4, BL, F], f32, tag="out_sb")
        part_sb = consts.tile([128, 8, 512], bf16, tag="x_in", name="part_sb")

        def chain_pt(idx):
            if idx % 2 == 0:
                return ps_misc.tile([128, 512], f32, tag="ps_misc",
                                    name="ps_out")
            return ps_proj.tile([128, 512], f32, tag="ps_proj", name="ps_out")

        for mt in range(4):
            for nck in range(NC2):
                idx = 2 * mt + nck
                pt = chain_pt(idx)
                k = 0
                for ch in range(2):
                    for ri, src in enumerate((re_sb, im_sb)):
                        nc.tensor.matmul(
                            out=pt,
                            lhsT=w_irfft_sb[:, ri, ch, mt, :],
                            rhs=src[:, ch, 512 * nck:512 * (nck + 1)],
                            start=(k == 0), stop=(k == 3),
                        )
                        k += 1
                if idx % 2 == 0:
                    nc.scalar.copy(out=part_sb[:, idx, :], in_=pt)
                else:
                    nc.vector.tensor_copy(out=part_sb[:, idx, :], in_=pt)
        for mt in range(4):
            for nck in range(NC2):
                idx = 2 * mt + nck
                pt = chain_pt(idx)
                for ri, src in enumerate((re_sb, im_sb)):
                    nc.tensor.matmul(
                        out=pt,
                        lhsT=w_irfft_sb[:, ri, 2, mt, :],
                        rhs=src[:, 2, 512 * nck:512 * (nck + 1)],
                        start=(ri == 0), stop=(ri == 1),
                    )
                nc.vector.tensor_add(
                    out=out_sb[:, mt, 4 * nck:4 * (nck + 1), :]
                        .rearrange("p b f -> p (b f)"),
                    in0=pt, in1=part_sb[:, idx, :])
                nc.gpsimd.dma_start(
                    out=out_d[:, mt, 4 * nck:4 * (nck + 1), :],
                    in_=out_sb[:, mt, 4 * nck:4 * (nck + 1), :])

    nc.finalize()
    return nc


_CACHE = {}


def _get_nc():
    if "nc" not in _CACHE:
        _CACHE["nc"] = build_kernel()
    return _CACHE["nc"]


def kernel(x, Wp, bp, Wg, bg, Wm, bm, Wph, bph, _trace=False):
    # host: cast to bf16 and pre-transpose per core into the SBUF layout
    # [128 tp, 4 kt, BL b, F f] so the x DMA is fully contiguous
    x = np.asarray(x, dtype=np.float32).astype(ml_dtypes.bfloat16)
    x = np.ascontiguousarray(
        x.reshape(B, 4, 128, F).transpose(2, 1, 0, 3))  # (128, 4, B, F)
    hw = build_host_weights(
        np.asarray(Wp, np.float32), np.asarray(bp, np.float32),
        np.asarray(Wg, np.float32), np.asarray(bg, np.float32),
        np.asarray(Wm, np.float32), np.asarray(bm, np.float32),
        np.asarray(Wph, np.float32), np.asarray(bph, np.float32),
    )
    nc = _get_nc()
    in_maps = []
    for i in range(NCORE):
        m = {"x": np.ascontiguousarray(x[:, :, i * BL:(i + 1) * BL])}
        m.update(hw)
        in_maps.append(m)
    res = run_bass_kernel_spmd(nc, in_maps, core_ids=list(range(NCORE)),
                               trace=_trace)
    # un-transpose: (128 p, 4 mt, BL b, F) -> (BL, H=mt*128+p, F)
    outs = []
    for r in res.results:
        o = np.asarray(r["out"])  # (128, 4, BL, F)
        outs.append(o.transpose(2, 1, 0, 3).reshape(BL, H, F))
    out = np.concatenate(outs, axis=0)
    if _trace:
        _CACHE["last_exec_time_ns"] = res.exec_time_ns
        _CACHE["last_results"] = res
    return out.astype(np.float32)

